# revision 46
# baseline (speedup 1.0000x reference)
"""Multi-head attention (B=4, N=2048, DIM=768, H=12) on 8 TRN2 cores.

Sharding: core c -> batch c//2, heads (c%2)*6 .. +6  (6 heads = 3 pairs).
Each core computes its heads' attention and a partial output projection
(row-sharded w_proj); host sums the two partials per batch and adds bias.

Per-core dataflow (bf16 matmuls, fp16 softmax weights/values):
  inputs : xt [768,2048] (= x[b].T), wq/wk/wv/wp host-premerged to
           [128, chunks*K] so each loads as ONE contiguous DMA (weights on
           the scalar HWDGE ring, xt as 12 pieces on the sync ring)
  qkv    : Q^T,K^T per head-pair [128,2048] (d-major), V token-major with a
           ones column appended per head ([64 V | 1] x 6 -> [128, 390])
  scores : S^T[keys, q], 2 heads row-tiled per key-tile matmul
  softmax: exp on ACT, fp16 out (scale folded in; max-subtraction skipped -
           scores O(5)); PV with M=65 makes row 64 of U the denominator for
           free; normalization fully off the PE: fast reciprocal (DVE) +
           gpsimd partition_broadcast + SBUF->SBUF DMA partition shift
  PV     : U^T[d+1, q] accumulated over key chunks (V' chunks stationary)
  proj   : partial = OT.T-slices @ wp (OT is d-major already), PSUM halves
           on the dr tag so proj weaves inside attention blocks

Schedule: the stage-1A Tile scheduler replays engines in order and pops
ready work greedily by emission priority, so all bulk PE filler (later
pairs' qkv, proj groups, the previous block's norm chain) is WOVEN between
the kt steps of attention blocks to keep the ACT exp stream (the ~211us
co-wall with the PE stream) continuously fed. The final block uses a
PE-based norm variant since the PE is idle at the tail and the gpsimd/DMA
hops would sit on the critical path.
"""

import sys

for _p in ("/opt/trn_rl_repo",):
    if _p not in sys.path:
        sys.path.insert(0, _p)

import numpy as np
import ml_dtypes

import concourse.bass as bass
import concourse.bacc as bacc
import concourse.mybir as mybir
import concourse.tile as tile
from concourse.bass_utils import run_bass_kernel_spmd
from concourse.masks import make_identity

DIM = 768
HEADS = 12
HD = 64
B = 4
N = 2048
NCORES = 8
PAIRS = 3          # head-pairs per core (6 heads)
CH = DIM // 128    # 6 contraction chunks of 128
KT = N // 128      # 16 key tiles
QB = N // 512      # 4 query blocks of 512
F32 = mybir.dt.float32
EXP = mybir.ActivationFunctionType.Exp
SCALE = HD ** -0.5

DEFAULT_DTYPE = "bf16"


def build_program(dtype="f32", debug=False, overlap=None):
    if overlap is None:
        overlap = dtype != "f32"
    dt = F32 if dtype == "f32" else mybir.dt.bfloat16
    nc = bacc.Bacc()
    # weights arrive host-premerged: w*[r, c*384+k] = W[c*128+r, k] so each
    # is ONE contiguous [128, 2304] DMA
    xt = nc.declare_dram_parameter("xt", [DIM, N], dt, isOutput=False)
    wq = nc.declare_dram_parameter("wq", [128, CH * PAIRS * 128], dt, isOutput=False)
    wk = nc.declare_dram_parameter("wk", [128, CH * PAIRS * 128], dt, isOutput=False)
    wv = nc.declare_dram_parameter("wv", [128, CH * PAIRS * 128], dt, isOutput=False)
    wp = nc.declare_dram_parameter("wp", [128, PAIRS * DIM], dt, isOutput=False)
    out = nc.declare_dram_parameter("out", [N, DIM], dt, isOutput=True)
    dbg = None
    if debug:
        dbg = {
            "dbg_v0": nc.declare_dram_parameter("dbg_v0", [128, 390], F32, isOutput=True),
            "dbg_den": nc.declare_dram_parameter("dbg_den", [1, 1024], F32, isOutput=True),
            "dbg_dsb": nc.declare_dram_parameter("dbg_dsb", [1, 1024], F32, isOutput=True),
            "dbg_e0": nc.declare_dram_parameter("dbg_e0", [128, 1024], F32, isOutput=True),
        }

    with tile.TileContext(nc) as tc:
        emit(tc, nc, xt, wq, wk, wv, wp, out, dt, overlap=overlap, dbg=dbg)
    nc.compile()
    return nc


def emit(tc, nc, xt, wq, wk, wv, wp, out, dt, overlap, dbg=None):
    import contextlib

    ctx = contextlib.ExitStack()
    wbufs = 2 if overlap else 1
    with ctx:
        sb = ctx.enter_context(tc.tile_pool(name="sb", bufs=1))
        ps = ctx.enter_context(tc.tile_pool(name="ps", bufs=1, space="PSUM"))

        # ---- load inputs -------------------------------------------------
        # DMA issue cost (~650ns/instr) on one HWDGE ring serializes: merge
        # each weight tensor into ONE strided DMA on the scalar ring, xt into
        # 12 contiguous [128,1024] pieces on the sync ring. Both rings issue
        # in parallel; Tile tracks deps at slice granularity.
        def load_w(dram, nm, parts=1):
            t = sb.tile([128, CH * PAIRS * 128], dt, name=nm, tag=nm)
            step = CH * PAIRS * 128 // parts
            for i in range(parts):
                nc.scalar.dma_start(out=t[:, i * step:(i + 1) * step],
                                    in_=dram[:, i * step:(i + 1) * step])
            return [t[:, ch * PAIRS * 128:(ch + 1) * PAIRS * 128]
                    for ch in range(CH)]

        # wq/wk halved so the first qkv groups unblock on the first half
        wq_sb = load_w(wq, "wq", parts=2)
        wk_sb = load_w(wk, "wk", parts=2)
        wv_sb = load_w(wv, "wv")

        xt_sb = [sb.tile([128, N], dt, name=f"xt{ch}", tag=f"xt{ch}")
                 for ch in range(CH)]
        for h in range(2):
            csl = slice(h * 1024, (h + 1) * 1024)
            for ch in range(CH):
                nc.sync.dma_start(out=xt_sb[ch][:, csl],
                                  in_=xt[ch * 128:(ch + 1) * 128, csl])

        wp_t = sb.tile([128, PAIRS * DIM], dt, name="wp", tag="wp")
        nc.scalar.dma_start(out=wp_t[:], in_=wp[:, :])
        wp_sb = [wp_t[:, ch * DIM:(ch + 1) * DIM] for ch in range(PAIRS)]

        ones_sb = sb.tile([128, 64], F32, name="ones", tag="ones")
        nc.vector.memset(ones_sb[:], 1.0)
        ident = sb.tile([128, 128], dt, name="ident", tag="ident")
        make_identity(nc, ident)

        # e/v (softmax weights and values) use fp16: exp output is in
        # [0, ~e^5] where fp16 beats bf16 precision, and it matmuls at the
        # same 1 cyc/col.
        edt = mybir.dt.float16 if dt != F32 else F32

        # persistent SBUF tensors
        # v' layout per head g: cols [g*65 .. g*65+63] = V, col g*65+64 = 1.0
        v_sb = [sb.tile([128, 6 * 65], edt, name=f"v{k}", tag=f"v{k}")
                for k in range(KT)]
        ot_sb = [sb.tile([128, N], dt, name=f"ot{p}", tag=f"ot{p}")
                 for p in range(PAIRS)]

        # ---- V' = [x @ wv | 1] (token-major) -----------------------------
        def emit_v(kt):
            pv = ps.tile([128, 512], F32, name="dr", tag="dr", bufs=2)
            for ch in range(CH):
                nc.tensor.matmul(
                    pv[:, :PAIRS * 128],
                    lhsT=xt_sb[ch][:, kt * 128:(kt + 1) * 128],
                    rhs=wv_sb[ch][:],
                    start=(ch == 0), stop=(ch == CH - 1),
                )
            v3 = v_sb[kt].rearrange("p (g c) -> p g c", c=65)
            p3 = pv[:, :PAIRS * 128].rearrange("p (g c) -> p g c", c=64)
            nc.vector.tensor_copy(v3[:, :, 0:64], p3[:])
            nc.vector.memset(v3[:, :, 64:65], 1.0)

        qt_tiles = {}
        kt_tiles = {}

        def emit_qkv_group(p, which, qb):
            """One accumulation group: 512 columns of Q^T or K^T for pair p."""
            w_sb, store, nm = (
                (wq_sb, qt_tiles, "qt") if which == 0 else (wk_sb, kt_tiles, "kt")
            )
            if qb == 0:
                store[p] = sb.tile([128, N], dt, name=f"{nm}{p}", tag=nm, bufs=wbufs)
            acc = ps.tile([128, 512], F32, name="dr", tag="dr", bufs=2)
            for ch in range(CH):
                nc.tensor.matmul(
                    acc[:],
                    lhsT=w_sb[ch][:, p * 128:(p + 1) * 128],
                    rhs=xt_sb[ch][:, qb * 512:(qb + 1) * 512],
                    start=(ch == 0), stop=(ch == CH - 1),
                )
            nc.vector.tensor_copy(store[p][:, qb * 512:(qb + 1) * 512], acc[:])

        def emit_proj_group(tt, tail=False):
            # two dr-sized PSUM halves so proj can interleave inside attn
            # blocks without competing for the "s" slots the exp stream
            # needs; the final tranche (no more scores) takes the free "s"
            # slots instead so it pipelines against the last norm chain.
            tsl = slice(tt * 128, (tt + 1) * 128)
            if tail:
                pp = ps.tile([128, 1024], F32, name="s", tag="s", bufs=2)
                pa, pb = pp[:, 0:512], pp[:, 512:768]
            else:
                pa = ps.tile([128, 512], F32, name="dr", tag="dr", bufs=2)
                pb = ps.tile([128, 256], F32, name="dr2", tag="dr", bufs=2)
            for ch in range(PAIRS):
                nc.tensor.matmul(
                    pa[:], lhsT=ot_sb[ch][:, tsl], rhs=wp_sb[ch][:, 0:512],
                    start=(ch == 0), stop=(ch == PAIRS - 1),
                )
                nc.tensor.matmul(
                    pb[:], lhsT=ot_sb[ch][:, tsl], rhs=wp_sb[ch][:, 512:768],
                    start=(ch == 0), stop=(ch == PAIRS - 1),
                )
            st = sb.tile([128, 768], dt, name="st", tag="st", bufs=3)
            if tail:
                # ACT is idle after the last exp: issue the final out stores
                # on the scalar HWDGE ring so they don't serialize behind the
                # sync ring's norm-path DMAs
                nc.vector.tensor_copy(st[:], pp[:, 0:768])
                nc.scalar.dma_start(out=out[tsl, :], in_=st[:])
            else:
                nc.vector.tensor_copy(st[:, 0:512], pa[:])
                nc.vector.tensor_copy(st[:, 512:768], pb[:])
                nc.sync.dma_start(out=out[tsl, :], in_=st[:])

        def attn_begin(p, qb):
            return {
                "p": p, "qb": qb,
                "qsl": slice(qb * 512, (qb + 1) * 512),
                "u_a": ps.tile([128, 512], F32, name="ua", tag="u", bufs=2),
                "u_b": ps.tile([128, 512], F32, name="ub", tag="u", bufs=2),
            }

        def attn_step(ast, kt):
            p, qsl = ast["p"], ast["qsl"]
            qt_t = qt_tiles[p]
            kt_t = kt_tiles[p]
            ksl = slice(kt * 128, (kt + 1) * 128)
            s_ps = ps.tile([128, 1024], F32, name="s", tag="s", bufs=2)
            # scores S^T for both heads, row-tiled (contract=64 each)
            nc.tensor.matmul(
                s_ps[:, 0:512],
                lhsT=kt_t[0:64, ksl], rhs=qt_t[0:64, qsl],
                start=True, stop=True,
            )
            nc.tensor.matmul(
                s_ps[:, 512:1024],
                lhsT=kt_t[64:128, ksl], rhs=qt_t[64:128, qsl],
                start=True, stop=True,
            )
            e_sb = sb.tile([128, 1024], edt, name="e", tag="e", bufs=3)
            nc.scalar.activation(e_sb[:], s_ps[:], EXP, scale=SCALE)
            first = kt == 0
            last = kt == KT - 1
            # PV with the ones column: U[0:64] = P@V, U[64] = denominator
            nc.tensor.matmul(
                ast["u_a"][0:65, :],
                lhsT=v_sb[kt][:, (2 * p) * 65:(2 * p) * 65 + 65],
                rhs=e_sb[:, 0:512],
                start=first, stop=last,
            )
            nc.tensor.matmul(
                ast["u_b"][0:65, :],
                lhsT=v_sb[kt][:, (2 * p + 1) * 65:(2 * p + 1) * 65 + 65],
                rhs=e_sb[:, 512:1024],
                start=first, stop=last,
            )

        def attn_end_copies(ast):
            # cheap copies so the U PSUM banks free up quickly; the heavier
            # normalization (attn_end_norm) is woven into the NEXT block
            u_a, u_b = ast["u_a"], ast["u_b"]
            dsb = sb.tile([65, 1024], F32, name="dsb", tag="dsb", bufs=2)
            nc.vector.tensor_copy(dsb[64:65, 0:512], u_a[64:65, :])
            nc.vector.tensor_copy(dsb[64:65, 512:1024], u_b[64:65, :])
            ua_sb = sb.tile([64, 512], dt, name="uasb", tag="uasb", bufs=2)
            nc.vector.tensor_copy(ua_sb[:], u_a[0:64, :])
            tmp = sb.tile([64, 512], dt, name="tmp", tag="tmp", bufs=2)
            nc.vector.tensor_copy(tmp[:], u_b[0:64, :])
            ast["dsb"], ast["ua_sb"], ast["tmp"] = dsb, ua_sb, tmp

        def attn_end_norm(ast):
            # normalization entirely off the PE: fast reciprocal of the
            # denominator row (DVE), partition-broadcast (GpSimd), multiply
            # (DVE); head 2's rows move to partitions 64-127 via a small
            # SBUF-to-SBUF DMA.
            p, qsl = ast["p"], ast["qsl"]
            dsb, ua_sb, tmp = ast["dsb"], ast["ua_sb"], ast["tmp"]
            # hop the denominator row to partition 0 (DMA), then broadcast:
            # partition_broadcast always reads its tile's partition 0
            den0 = sb.tile([1, 1024], F32, name="den0", tag="den0", bufs=2)
            nc.sync.dma_start(out=den0[:], in_=dsb[64:65, :])
            den = sb.tile([128, 1024], F32, name="den", tag="den", bufs=2)
            nc.gpsimd.partition_broadcast(den[:], den0[0:1, :])
            rsb = sb.tile([128, 1024], F32, name="rsb", tag="rsb", bufs=2)
            nc.vector.reciprocal_approx_fast(out=rsb[:], in_=den[:])
            nc.vector.tensor_mul(ot_sb[p][0:64, qsl], ua_sb[:], rsb[0:64, 0:512])
            tmp2 = sb.tile([64, 512], dt, name="tmp2", tag="tmp2", bufs=2)
            nc.vector.tensor_mul(tmp2[:], tmp[:], rsb[0:64, 512:1024])
            nc.sync.dma_start(out=ot_sb[p][64:128, qsl], in_=tmp2[:])

        def attn_end_norm_tail(ast):
            # final-block variant: PE is idle at the tail, and the
            # DMA+broadcast hops cost ~5us of un-hidden latency there, so
            # replicate denominators and shift head 2 on the PE instead.
            p, qsl = ast["p"], ast["qsl"]
            dsb, ua_sb, tmp = ast["dsb"], ast["ua_sb"], ast["tmp"]
            r_ps = ps.tile([128, 512], F32, name="dr", tag="dr", bufs=2)
            nc.tensor.matmul(
                r_ps[0:64, :], lhsT=ones_sb[64:65, 0:64], rhs=dsb[64:65, 0:512],
                start=True, stop=True,
            )
            nc.tensor.matmul(
                r_ps[64:128, :], lhsT=ones_sb[64:65, 0:64], rhs=dsb[64:65, 512:1024],
                start=True, stop=True,
            )
            rsb = sb.tile([128, 1024], F32, name="rsb", tag="rsb", bufs=2)
            nc.vector.reciprocal_approx_fast(out=rsb[:, 0:512], in_=r_ps[:])
            o2 = ps.tile([128, 512], F32, name="dr", tag="dr", bufs=2)
            nc.tensor.matmul(
                o2[64:128, :], lhsT=ident[0:64, 0:64], rhs=tmp[:],
                start=True, stop=True,
            )
            nc.vector.tensor_mul(ot_sb[p][0:64, qsl], ua_sb[:], rsb[0:64, 0:512])
            nc.vector.tensor_mul(ot_sb[p][64:128, qsl], o2[64:128, :],
                                 rsb[64:128, 0:512])

        def attn_end(ast):
            attn_end_copies(ast)
            attn_end_norm(ast)

        def attn_block(p, qb, fillers=(), prev=None):
            """One attention block with PE filler work woven BETWEEN kt
            steps, so fillers land in the PE's exp-wait slack instead of
            running as a bulk slug that starves the ACT stream. The previous
            block's norm chain runs as the first filler."""
            fillers = list(fillers)
            nf = len(fillers)
            pos = [((i + 1) * KT) // (nf + 1) for i in range(nf)]
            ast = attn_begin(p, qb)
            fi = 0
            for kt in range(KT):
                attn_step(ast, kt)
                if kt == 0 and prev is not None:
                    attn_end_norm(prev)
                while fi < nf and pos[fi] <= kt:
                    fillers[fi]()
                    fi += 1
            attn_end_copies(ast)
            while fi < nf:
                fillers[fi]()
                fi += 1
            return ast

        def emit_attn_qb(p, qb):
            ast = attn_begin(p, qb)
            for kt in range(KT):
                attn_step(ast, kt)
            attn_end(ast)

        # ---- schedule ----------------------------------------------------
        if overlap:
            # Slim pipelined head: only Q0(qb0)/K0(qb0)/V'(0:2) precede the
            # first attention steps; remaining K0/Q0/V' weave into the first
            # block so the ACT exp stream starts as early as possible. Later
            # pairs' qkv groups and proj groups weave between the kt steps of
            # subsequent blocks; each block's norm chain runs inside the next.
            emit_qkv_group(0, 0, 0)
            emit_qkv_group(0, 1, 0)
            for kt in range(2):
                emit_v(kt)
            ast0 = attn_begin(0, 0)
            for kt in range(KT):
                if kt + 2 < KT:
                    emit_v(kt + 2)
                if kt % 4 == 1 and kt // 4 < 3:
                    emit_qkv_group(0, 1, kt // 4 + 1)
                attn_step(ast0, kt)
                if kt in (6, 10, 14):
                    emit_qkv_group(0, 0, (kt - 2) // 4)
            attn_end_copies(ast0)
            prev = ast0
            # pair p+1's 8 qkv groups spread over pair p's attn blocks
            g1 = [(1, w, qb) for w in (0, 1) for qb in range(QB)]
            for i, qb in enumerate(range(1, QB)):
                gs = g1[3 * i:3 * i + 3]
                prev = attn_block(0, qb,
                                  [lambda g=g: emit_qkv_group(*g) for g in gs],
                                  prev=prev)
            g2 = [(2, w, qb) for w in (0, 1) for qb in range(QB)]
            for qb in range(QB):
                gs = g2[2 * qb:2 * qb + 2]
                prev = attn_block(1, qb,
                                  [lambda g=g: emit_qkv_group(*g) for g in gs],
                                  prev=prev)
            prev = attn_block(2, 0, prev=prev)
            for qb in range(1, QB):
                prev = attn_block(2, qb, [lambda t=tt: emit_proj_group(t)
                                          for tt in range(4 * (qb - 1), 4 * qb)],
                                  prev=prev)
            attn_end_norm_tail(prev)
            for tt in range(4 * (QB - 1), 4 * QB):
                emit_proj_group(tt, tail=True)
        else:
            for kt in range(KT):
                emit_v(kt)
            for p in range(PAIRS):
                for which in (0, 1):
                    for qb in range(QB):
                        emit_qkv_group(p, which, qb)
                for qb in range(QB):
                    emit_attn_qb(p, qb)
            for tt in range(KT):
                emit_proj_group(tt)


_NC = {}


def _get_nc(dtype, overlap=None):
    key = (dtype, overlap)
    if key not in _NC:
        _NC[key] = build_program(dtype, overlap=overlap)
    return _NC[key]


def make_in_maps(x, w_qkv, w_proj, dtype):
    np_dt = np.float32 if dtype == "f32" else ml_dtypes.bfloat16
    def merge(w):
        # [CH*128, K] -> [128, CH*K]: row r gets all chunks side by side
        chn = w.shape[0] // 128
        return np.ascontiguousarray(
            w.reshape(chn, 128, w.shape[1]).transpose(1, 0, 2)
            .reshape(128, chn * w.shape[1]))

    in_maps = []
    for c in range(NCORES):
        b = c // 2
        h0 = (c % 2) * 6 * HD
        in_maps.append({
            "xt": np.ascontiguousarray(x[b].T).astype(np_dt),
            "wq": merge(w_qkv[:, h0:h0 + 384]).astype(np_dt),
            "wk": merge(w_qkv[:, DIM + h0:DIM + h0 + 384]).astype(np_dt),
            "wv": merge(w_qkv[:, 2 * DIM + h0:2 * DIM + h0 + 384]).astype(np_dt),
            "wp": merge(w_proj[h0:h0 + 384, :]).astype(np_dt),
        })
    return in_maps


def run(x, w_qkv, w_proj, b_proj, trace=False, dtype=None, overlap=None):
    dtype = dtype or DEFAULT_DTYPE
    x = np.asarray(x, dtype=np.float32)
    w_qkv = np.asarray(w_qkv, dtype=np.float32)
    w_proj = np.asarray(w_proj, dtype=np.float32)
    b_proj = np.asarray(b_proj, dtype=np.float32)

    in_maps = make_in_maps(x, w_qkv, w_proj, dtype)
    res = run_bass_kernel_spmd(_get_nc(dtype, overlap), in_maps, list(range(NCORES)),
                               trace=trace)
    full = np.empty((B, N, DIM), dtype=np.float32)
    for b in range(B):
        full[b] = (res.results[2 * b]["out"].astype(np.float32)
                   + res.results[2 * b + 1]["out"].astype(np.float32) + b_proj)
    return full, res


def kernel(x, w_qkv, w_proj, b_proj):
    full, _ = run(x, w_qkv, w_proj, b_proj, trace=False)
    return full

